# revision 20
# baseline (speedup 1.0000x reference)
"""Trainium2 Bass kernel for BalancedFrequencyAttention.

Math: the reference does DCT(W) -> frequency split/mix -> IDCT -> GAP -> tiny
SE gate -> x * att.  Everything from x to gap is linear, so gap[b,c] ==
sum_{h,n} x[b,c,h,n] * G[h,n] for a fixed matrix G.  G turns out to have only
4 distinct rows: one pattern for h < 80 and a period-3 family for h >= 80.
So the kernel is a memory-bound two-pass streaming kernel:
  pass 1: h-class partial sums of x (VectorE adds), then 4 tiny dot products
  MLP:    att = sigmoid(w2 @ (w1 @ gap))                        (TensorE)
  pass 2: out = x * att[c]                                      (per-partition scale)
Sharding: batch b=8 -> one sample per NeuronCore, no cross-core communication.
"""

import sys

if "/opt/trn_rl_repo" not in sys.path:
    sys.path.insert(0, "/opt/trn_rl_repo")

import numpy as np

B, C, H, W = 8, 128, 200, 480
K = W // 2          # 240: balanced-spectrum width
HS = 80             # H - W//4: high-freq start row
N_CORES = 8

# pass-1 order: hi streamed, hi cached (stay in SBUF for pass 2), low last
# (small final tiles keep the att critical path short).
N_CACHE = 5
CACHE_START = H - N_CACHE * 12                            # rows >= 140 cached
HI_STREAM = [(h0, 12) for h0 in range(HS, CACHE_START, 12)]
HI_CACHED = [(h0, 12) for h0 in range(CACHE_START, H, 12)]
LOW_TILES = [(h0, 8) for h0 in range(0, 72, 8)] + [(72, 6), (78, 2)]

STORE_RING = "sync"     # "scalar" | "sync": ring for pass-2 stores
P2_SCALAR_MUL = True    # route 1/3 of pass-2 multiplies to ScalarE compute
P2_ROWS = 12            # pass-2 streamed tile height
STREAM_BUFS = 3

_nc_cache = None
_const_cache = None


def _build_constants():
    """Fold the whole DCT pipeline into per-row weight vectors (float64)."""

    def dct_mat(n):
        i = np.arange(n, dtype=np.float64)
        m = np.cos(np.pi * (2.0 * i[None, :] + 1.0) * i[:, None] / (2.0 * n)) * np.sqrt(2.0 / n)
        m[0] *= 1.0 / np.sqrt(2.0)
        return m

    D = dct_mat(W)
    D2 = dct_mat(K)
    s = D2.sum(axis=1)                     # row sums of the IDCT matrix
    t = s @ D[:K, :]                       # low-band weight profile, [W]
    alpha = 0.6 / (H * K)
    beta = 0.4 / (H * K)
    # high[b,c,h',k'] = x_dct[:, :, 80+f//400, 80+f%400], f = h'*240+k'; the
    # s-weight index (f mod 240) depends on h only through (h-80)%3.
    offs = [0, 160, 80]
    V = np.stack(
        [s[(offs[j] + np.arange(HS, W) - HS) % K] @ D[HS:W, :] for j in range(3)]
    )
    g_low = (alpha * t).astype(np.float32)                                  # [480]
    g_hi = np.concatenate([alpha * t + beta * V[j] for j in range(3)]).astype(np.float32)  # [1440]
    g_low_rep = np.ascontiguousarray(np.broadcast_to(g_low, (128, W)))
    g_hi_rep = np.ascontiguousarray(np.broadcast_to(g_hi, (128, 3 * W)))
    return g_low_rep, g_hi_rep


def _build_kernel():
    import concourse.bacc as bacc
    import concourse.tile as tile
    from concourse import mybir

    f32 = mybir.dt.float32
    nc = bacc.Bacc("TRN2", target_bir_lowering=False, debug=False, num_devices=N_CORES)

    x = nc.dram_tensor("x", [C, H, W], f32, kind="ExternalInput")
    g_low = nc.dram_tensor("g_low", [128, W], f32, kind="ExternalInput")
    g_hi = nc.dram_tensor("g_hi", [128, 3 * W], f32, kind="ExternalInput")
    w1t = nc.dram_tensor("w1t", [C, C // 4], f32, kind="ExternalInput")
    w2t = nc.dram_tensor("w2t", [C // 4, C], f32, kind="ExternalInput")
    out = nc.dram_tensor("out", [C, H, W], f32, kind="ExternalOutput")

    add = mybir.AluOpType.add
    X = mybir.AxisListType.X

    def dram_rows(t, h0, rows):
        return t[:, h0 : h0 + rows, :].rearrange("p r w -> p (r w)")

    with tile.TileContext(nc) as tc:
        with (
            tc.tile_pool(name="stream", bufs=STREAM_BUFS) as stream,
            tc.tile_pool(name="cachep", bufs=N_CACHE) as cachep,
            tc.tile_pool(name="consts", bufs=1) as consts,
            tc.tile_pool(name="small", bufs=1) as small,
            tc.tile_pool(name="psum", bufs=1, space="PSUM") as psum,
        ):
            g_low_t = consts.tile([128, W], f32, tag="g_low")
            nc.sync.dma_start(g_low_t[:], g_low[:])
            g_hi_t = consts.tile([128, 3 * W], f32, tag="g_hi")
            nc.sync.dma_start(g_hi_t[:], g_hi[:])
            w1t_t = consts.tile([C, C // 4], f32, tag="w1t")
            nc.sync.dma_start(w1t_t[:], w1t[:])
            w2t_t = consts.tile([C // 4, C], f32, tag="w2t")
            nc.sync.dma_start(w2t_t[:], w2t[:])
            acc_hi = consts.tile([128, 6 * W], f32, tag="acc_hi")  # 2x (3-row classes)
            acc_lo = consts.tile([128, 2 * W], f32, tag="acc_lo")  # 2x low rows
            nc.vector.memset(acc_hi[:], 0.0)
            nc.vector.memset(acc_lo[:], 0.0)
            partials = small.tile([128, 2], f32, tag="partials")
            att = small.tile([128, 1], f32, tag="att")
            # preload the sigmoid LUT off the critical path
            nc.gpsimd.memset(att[:], 0.0)
            nc.scalar.activation(att[:], att[:], mybir.ActivationFunctionType.Sigmoid)

            # ---- pass 1: stream x, accumulate h-class sums ----
            cached_tiles = []

            def p1_tile(h0, rows, cached):
                pool = cachep if cached else stream
                xt = pool.tile([128, rows * W], f32, tag="cache" if cached else "xs")
                nc.sync.dma_start(xt[:], dram_rows(x, h0, rows))
                acc, grp = (acc_lo, 2 * W) if h0 < HS else (acc_hi, 6 * W)
                for k in range(rows * W // grp):
                    nc.vector.tensor_add(acc[:], acc[:], xt[:, k * grp : (k + 1) * grp])
                if cached:
                    cached_tiles.append((xt, h0, rows))

            for h0, rows in HI_STREAM:
                p1_tile(h0, rows, False)
            for h0, rows in LOW_TILES:
                p1_tile(h0, rows, False)
            # low-class dot product: runs mid-kernel, off the critical path
            nc.vector.tensor_add(acc_lo[:, :W], acc_lo[:, :W], acc_lo[:, W:])
            nc.vector.tensor_mul(acc_lo[:, W:], acc_lo[:, :W], g_low_t[:])
            nc.vector.tensor_reduce(partials[:, 1:2], acc_lo[:, W:], axis=X, op=add)
            # cached tiles last: dedicated slots, so the pass-1 tail never
            # waits on DVE-paced slot recycling
            for h0, rows in HI_CACHED:
                p1_tile(h0, rows, True)
            nc.vector.tensor_add(acc_hi[:, : 3 * W], acc_hi[:, : 3 * W], acc_hi[:, 3 * W :])
            nc.vector.tensor_mul(acc_hi[:, 3 * W :], acc_hi[:, : 3 * W], g_hi_t[:])
            nc.vector.tensor_reduce(partials[:, 0:1], acc_hi[:, 3 * W :], axis=X, op=add)

            # ---- gap -> SE MLP -> att ----
            gap = small.tile([128, 1], f32, tag="gap")
            nc.vector.tensor_reduce(gap[:], partials[:], axis=X, op=add)
            y_p = psum.tile([C // 4, 1], f32, tag="y_p")
            nc.tensor.matmul(y_p[:], w1t_t[:], gap[:], start=True, stop=True)
            y_s = small.tile([C // 4, 1], f32, tag="y_s")
            nc.vector.tensor_copy(y_s[:], y_p[:])
            z_p = psum.tile([C, 1], f32, tag="z_p")
            nc.tensor.matmul(z_p[:], w2t_t[:], y_s[:], start=True, stop=True)
            nc.scalar.activation(att[:], z_p[:], mybir.ActivationFunctionType.Sigmoid)

            # ---- pass 2: out = x * att ----
            # Software-pipelined issue order: keep STREAM_BUFS loads issued
            # ahead of the first (att-gated) store so the sequencer's blocking
            # wait leaves the DMA engines with queued work during the MLP.
            store_engine = nc.scalar if STORE_RING == "scalar" else nc.sync
            p2 = [
                (h0, min(P2_ROWS, CACHE_START - h0))
                for h0 in range(0, CACHE_START, P2_ROWS)
            ]
            pending = []

            def p2_flush(i, xt, h0, rows):
                if P2_SCALAR_MUL and i % 3 == 2 and STORE_RING != "scalar":
                    nc.scalar.mul(xt[:], xt[:], att[:, 0:1])
                else:
                    nc.vector.tensor_scalar_mul(xt[:], xt[:], att[:, 0:1])
                store_engine.dma_start(dram_rows(out, h0, rows), xt[:])

            for i, (h0, rows) in enumerate(p2):
                xt = stream.tile([128, rows * W], f32, tag="xs")
                nc.sync.dma_start(xt[:], dram_rows(x, h0, rows))
                pending.append((i, xt, h0, rows))
                if len(pending) >= STREAM_BUFS:
                    p2_flush(*pending.pop(0))
            for item in pending:
                p2_flush(*item)
            # cached rows (already on-chip)
            for xt, h0, rows in cached_tiles:
                nc.vector.tensor_scalar_mul(xt[:], xt[:], att[:, 0:1])
                store_engine.dma_start(dram_rows(out, h0, rows), xt[:])

    nc.compile()
    return nc


def _get_compiled():
    global _nc_cache, _const_cache
    if _nc_cache is None:
        _nc_cache = _build_kernel()
        _const_cache = _build_constants()
    return _nc_cache, _const_cache


def kernel(x, w1, w2, **_unused):
    from concourse.bass_utils import run_bass_kernel_spmd

    nc, (g_low_rep, g_hi_rep) = _get_compiled()
    x = np.ascontiguousarray(np.asarray(x), dtype=np.float32)
    w1t = np.ascontiguousarray(np.asarray(w1, dtype=np.float32).T)
    w2t = np.ascontiguousarray(np.asarray(w2, dtype=np.float32).T)
    in_maps = [
        {
            "x": np.ascontiguousarray(x[i]),
            "g_low": g_low_rep,
            "g_hi": g_hi_rep,
            "w1t": w1t,
            "w2t": w2t,
        }
        for i in range(N_CORES)
    ]
    res = run_bass_kernel_spmd(nc, in_maps, list(range(N_CORES)))
    outs = [np.asarray(r["out"], dtype=np.float32) for r in res.results]
    return np.stack(outs, axis=0)
